# revision 1
# baseline (speedup 1.0000x reference)
"""AssociativeEmbeddingLoss on 8 TRN2 NeuronCores (Bass/Tile kernel).

Entry point: kernel(**inputs) -> np.ndarray of shape (3,) =
(pull_loss, push_loss, scale_loss), matching the reference.

Sharding: data-parallel on batch dim N=16 -> 2 images per core
(tags/joints/box_scales sharded on dim 0, scale_dist replicated); the
final three scalar means are reduced on the host from the per-image
partials each core returns (the "all-reduced means" gather step).

Per-core kernel design:
  - The loss touches tags only at 60 persons x 17 joints gathered rows
    of 16 floats, so instead of streaming the 8.9MB tags shard the
    kernel indirect-DMA-gathers just those ~65KB.
  - Indirect DMA costs ~1.4us per CALL (one offset per partition), so
    two joint columns are packed per call on 128 partitions (persons
    duplicated at partition offset 64): 9 calls instead of 17.
  - Partition halves merge via one PE matmul against a 0/1 selector.
  - Everything not dependent on the gathered tags (visibility counts,
    reciprocals, per-image n_val chain, box-scale target norms) runs
    hidden under the gather stream; the gathered-data statistics are
    chunked so DVE overlaps the remaining gather calls.
  - The pairwise push term is one 18x60 @ 18x60 PE matmul + Exp
    activation, with invalid persons masked by a +BIG additive term
    folded into the feature vectors (exp(-BIG) == 0).
"""

import numpy as np

import concourse.bacc as bacc
import concourse.mybir as mybir
import concourse.tile as tile
from concourse.bass import IndirectOffsetOnAxis
from concourse.bass_utils import run_bass_kernel_spmd

F32 = mybir.dt.float32
I32 = mybir.dt.int32
AF = mybir.ActivationFunctionType

S = 16  # scale-embedding dim
K = 17  # joints
M = 30  # persons per image
N = 16  # batch
L = 69632  # flattened tag locations per image (17*256*256/16)
N_CORES = 8
N_IMG = N // N_CORES  # images per core
BIG = 1e4


def _consts_np(n_img):
    J = n_img * M
    ident = np.eye(J, dtype=np.float32)
    img = np.arange(J) // M
    maskp = (img[:, None] == img[None, :]).astype(np.float32) - np.eye(J, dtype=np.float32)
    maskp = np.maximum(maskp, 0.0)
    maskp = (BIG / 2) * (1.0 - maskp)  # additive: exp(-2*x) kills masked pairs
    ind2 = np.zeros((J, n_img), np.float32)
    ind2[np.arange(J), img] = 1.0
    blob = np.concatenate([ident, maskp, ind2], axis=1)  # [J, 2J+n_img]
    # doubled-partition image offsets: persons at partitions 0..J-1 and TOP..TOP+J-1
    offs2 = np.zeros((128, 1), np.int32)
    offs2[0:J, 0] = img * L
    offs2[64 : 64 + J, 0] = img * L
    # half-merge selector: out[j] = in[j] + in[64+j] via PE
    sel = np.zeros((128, J), np.float32)
    sel[np.arange(J), np.arange(J)] = 1.0
    sel[64 + np.arange(J), np.arange(J)] = 1.0
    return blob, offs2, sel


def build_nc(n_img=2):
    J = n_img * M  # persons per core (60)
    P2 = 128  # doubled partition space; top half starts at TOP (engine APs
    TOP = 64  # may only start at partitions 0/32/64/96)
    KB = 9  # joint columns per partition half (ceil(17/2))

    nc = bacc.Bacc("TRN2", target_bir_lowering=False, debug=False)

    tags = nc.dram_tensor("tags", [n_img * L, S], F32, kind="ExternalInput")
    joints = nc.dram_tensor("joints", [128, 2 * KB], I32, kind="ExternalInput")
    bs = nc.dram_tensor("bs", [J, 1 + S], F32, kind="ExternalInput")
    out = nc.dram_tensor("out", [n_img, 3], F32, kind="ExternalOutput")

    blob_np, offs2_np, sel_np = _consts_np(n_img)
    blob_d = nc.inline_tensor(blob_np, "blob_c")
    sel_d = nc.inline_tensor(sel_np, "sel_c")

    with tile.TileContext(nc) as tc:
        with (
            tc.tile_pool(name="sb", bufs=1) as sb,
            tc.tile_pool(name="ps", bufs=1, space="PSUM") as ps,
        ):
            # ---- ACT table preloads (Abs/Sqrt/Exp) off the critical path ----
            warm = sb.tile([1, 1], F32, tag="warm")
            nc.vector.memset(warm[:], 1.0)
            w2 = sb.tile([1, 3], F32, tag="w2")
            nc.scalar.activation(out=w2[:, 0:1], in_=warm[:], func=AF.Abs)
            nc.scalar.activation(out=w2[:, 1:2], in_=warm[:], func=AF.Sqrt)
            nc.scalar.activation(out=w2[:, 2:3], in_=warm[:], func=AF.Exp)

            # ---- loads ----
            # joints arrive pre-packed by the host sharding step as [128,18]
            # int32: partitions 0-59 = (loc,vis) pairs of joints 0-8,
            # partitions 64-123 = joints 9-16, loc already rebased into the
            # core's [2L,16] shard view; pad partitions/cols are zero.
            j2 = sb.tile([P2, 2 * KB], I32, tag="j2")
            nc.sync.dma_start(j2[:], joints.ap())

            sel_sb = sb.tile([P2, J], F32, tag="sel_sb")
            nc.scalar.dma_start(sel_sb[:], sel_d.ap())
            blob_sb = sb.tile([J, 2 * J + n_img], F32, tag="blob_sb")
            nc.scalar.dma_start(blob_sb[:], blob_d.ap())
            bs_sb = sb.tile([J, 1 + S], F32, tag="bs_sb")
            nc.scalar.dma_start(bs_sb[:], bs.ap())

            ident_sb = blob_sb[:, 0:J]
            maskp_sb = blob_sb[:, J : 2 * J]
            ind2_sb = blob_sb[:, 2 * J : 2 * J + n_img]
            box_sb = bs_sb[:, 0:1]
            sd_sb = bs_sb[:, 1 : 1 + S]

            # ---- visibility ----
            j2v = j2[:].rearrange("p (k c) -> p k c", c=2)
            visf16 = sb.tile([P2, KB * S], F32, tag="visf16")
            nc.vector.tensor_copy(
                out=visf16[:].rearrange("p (k s) -> p k s", s=S),
                in_=j2v[:, :, 1:2].to_broadcast([P2, KB, S]),
            )

            # ---- the gather ----
            G = sb.tile([P2, KB * S], F32, tag="G")
            nc.vector.memset(G[:, (KB - 1) * S : KB * S], 0.0)
            for t in [KB - 1] + list(range(KB - 1)):
                pc = P2 if t < KB - 1 else J
                nc.gpsimd.indirect_dma_start(
                    out=G[0:pc, t * S : (t + 1) * S],
                    out_offset=None,
                    in_=tags.ap(),
                    in_offset=IndirectOffsetOnAxis(
                        ap=j2[0:pc, 2 * t : 2 * t + 1], axis=0
                    ),
                )

            # ---- early chains hidden under the gather ----
            # visibility counts -> safe_cnt/recip/valid and the per-image
            # n_val finalize chain, all independent of the gathered data.
            c16b = sb.tile([P2, 1], F32, tag="c16b")
            nc.vector.reduce_sum(out=c16b[:], in_=visf16[:], axis=mybir.AxisListType.X)
            cm_ps = ps.tile([J, 1], F32, tag="cm_ps")
            nc.tensor.matmul(
                out=cm_ps[:], lhsT=sel_sb[:], rhs=c16b[:], start=True, stop=True
            )
            c16e = sb.tile([J, 1], F32, tag="c16e")
            nc.vector.tensor_copy(out=c16e[:], in_=cm_ps[:])
            safe_cnt = sb.tile([J, 1], F32, tag="safe_cnt")
            nc.vector.tensor_scalar(
                out=safe_cnt[:], in0=c16e[:], scalar1=1.0 / S, scalar2=1.0,
                op0=mybir.AluOpType.mult, op1=mybir.AluOpType.max,
            )
            recip = sb.tile([J, 1], F32, tag="recip")
            nc.vector.reciprocal(out=recip[:], in_=safe_cnt[:])
            valid = sb.tile([J, 1], F32, tag="valid")
            nc.vector.tensor_single_scalar(
                out=valid[:], in_=c16e[:], scalar=0.5 * S, op=mybir.AluOpType.is_ge
            )
            nr = sb.tile([J, 1], F32, tag="nr")
            nc.vector.tensor_scalar_mul(out=nr[:], in0=recip[:], scalar1=-1.0)
            recip2 = sb.tile([J, 1], F32, tag="recip2")
            nc.vector.tensor_mul(out=recip2[:], in0=recip[:], in1=recip[:])
            rrv0 = sb.tile([J, 1], F32, tag="rrv0")
            nc.vector.tensor_scalar_mul(out=rrv0[:], in0=recip[:], scalar1=1.0 / S)
            rrv = sb.tile([J, 1], F32, tag="rrv")
            nc.vector.tensor_mul(out=rrv[:], in0=rrv0[:], in1=valid[:])
            hv = sb.tile([J, 1], F32, tag="hv")
            nc.vector.tensor_scalar(
                out=hv[:], in0=valid[:], scalar1=-BIG / 2, scalar2=BIG / 2,
                op0=mybir.AluOpType.mult, op1=mybir.AluOpType.add,
            )

            nv_ps = ps.tile([n_img, 1], F32, tag="nv_ps")
            nc.tensor.matmul(
                out=nv_ps[:], lhsT=ind2_sb, rhs=valid[:], start=True, stop=True
            )
            nvs = sb.tile([n_img, 1], F32, tag="nvs")
            nc.vector.tensor_copy(out=nvs[:], in_=nv_ps[:])
            safe_n = sb.tile([n_img, 1], F32, tag="safe_n")
            nc.vector.tensor_scalar_max(out=safe_n[:], in0=nvs[:], scalar1=1.0)
            rn = sb.tile([n_img, 1], F32, tag="rn")
            nc.vector.reciprocal(out=rn[:], in_=safe_n[:])
            nm1 = sb.tile([n_img, 1], F32, tag="nm1")
            nc.vector.tensor_scalar_add(out=nm1[:], in0=nvs[:], scalar1=-1.0)
            ppm = sb.tile([n_img, 1], F32, tag="ppm")
            nc.vector.tensor_scalar(
                out=ppm[:], in0=nvs[:], scalar1=nm1[:], scalar2=1.0,
                op0=mybir.AluOpType.mult, op1=mybir.AluOpType.max,
            )
            rp = sb.tile([n_img, 1], F32, tag="rp")
            nc.vector.reciprocal(out=rp[:], in_=ppm[:])
            ge2 = sb.tile([n_img, 1], F32, tag="ge2")
            nc.vector.tensor_single_scalar(
                out=ge2[:], in_=nvs[:], scalar=1.5, op=mybir.AluOpType.is_ge
            )

            # scale-loss pieces that depend only on box/scale_dist
            d0 = sb.tile([J, S], F32, tag="d0")
            nc.vector.tensor_scalar(
                out=d0[:], in0=sd_sb, scalar1=box_sb, scalar2=None,
                op0=mybir.AluOpType.subtract,
            )
            gap = sb.tile([J, S], F32, tag="gap")
            nc.scalar.activation(out=gap[:], in_=d0[:], func=AF.Abs)
            gap_e = sb.tile([J, S], F32, tag="gap_e")
            nc.vector.tensor_scalar_add(out=gap_e[:], in0=gap[:], scalar1=1e-10)
            r = sb.tile([J, S], F32, tag="r")
            nc.vector.reciprocal(out=r[:], in_=gap_e[:])
            r2 = sb.tile([J, S], F32, tag="r2")
            B2 = sb.tile([J, 1], F32, tag="B2")
            nc.vector.tensor_mul(out=r2[:], in0=r[:], in1=r[:])
            nc.vector.reduce_sum(out=B2[:], in_=r2[:], axis=mybir.AxisListType.X)
            mB = sb.tile([J, 1], F32, tag="mB")
            nc.vector.tensor_scalar_max(out=mB[:], in0=B2[:], scalar1=1e-24)
            sB = sb.tile([J, 1], F32, tag="sB")
            nc.scalar.sqrt(out=sB[:], in_=mB[:])
            rB = sb.tile([J, 1], F32, tag="rB")
            nc.vector.reciprocal(out=rB[:], in_=sB[:])

            # push feature tiles whose constant columns can fill early
            Wa = sb.tile([J, 18], F32, tag="Wa")
            nc.vector.memset(Wa[:, S + 1 : S + 2], 1.0)
            Wb = sb.tile([J, 18], F32, tag="Wb")
            nc.vector.memset(Wb[:, S : S + 1], 1.0)

            # ---- per-person stats, chunked to overlap the gather ----
            # gather order is [t8, t0..t7]; process block 8 first, then
            # blocks 0-3, 4-6, and finally just block 7 so only one block's
            # worth of DVE work trails the last gather call.
            gvb = sb.tile([P2, KB * S], F32, tag="gvb")
            gq = sb.tile([P2, KB * S], F32, tag="gq")
            UA = sb.tile([P2, S], F32, tag="UA")
            UB = sb.tile([P2, S], F32, tag="UB")
            sg = sb.tile([P2, 4], F32, tag="sg")
            tA = sb.tile([P2, 2 * S], F32, tag="tA")
            tB1 = sb.tile([P2, S], F32, tag="tB1")
            for ci, (lo, hi) in [(3, (8, 9)), (0, (0, 4)), (1, (4, 7)), (2, (7, 8))]:
                sl = slice(lo * S, hi * S)
                nc.vector.tensor_mul(out=gvb[:, sl], in0=G[:, sl], in1=visf16[:, sl])
                nc.vector.tensor_mul(out=gq[:, sl], in0=gvb[:, sl], in1=gvb[:, sl])
                nc.vector.reduce_sum(
                    out=sg[:, ci : ci + 1], in_=gq[:, sl], axis=mybir.AxisListType.X
                )
                if ci == 0:
                    nc.vector.tensor_add(
                        out=tA[:], in0=gvb[:, 0 : 2 * S], in1=gvb[:, 2 * S : 4 * S]
                    )
                    nc.vector.tensor_add(
                        out=UA[:], in0=tA[:, 0:S], in1=tA[:, S : 2 * S]
                    )
                elif ci == 1:
                    nc.vector.tensor_add(
                        out=tB1[:], in0=gvb[:, 4 * S : 5 * S], in1=gvb[:, 5 * S : 6 * S]
                    )
                    nc.vector.tensor_add(
                        out=UB[:], in0=tB1[:], in1=gvb[:, 6 * S : 7 * S]
                    )

            # Pack U | sum(g^2) on 128 partitions, merge halves via PE selector.
            Hpack = sb.tile([P2, S + 1], F32, tag="Hpack")
            UAB = sb.tile([P2, S], F32, tag="UAB")
            nc.vector.tensor_add(out=UAB[:], in0=UA[:], in1=UB[:])
            U78 = sb.tile([P2, S], F32, tag="U78")
            nc.vector.tensor_add(
                out=U78[:], in0=gvb[:, 7 * S : 8 * S], in1=gvb[:, 8 * S : 9 * S]
            )
            nc.vector.tensor_add(out=Hpack[:, 0:S], in0=UAB[:], in1=U78[:])
            nc.vector.reduce_sum(
                out=Hpack[:, S : S + 1], in_=sg[:], axis=mybir.AxisListType.X
            )
            Hm = ps.tile([J, S + 1], F32, tag="Hm")
            nc.tensor.matmul(
                out=Hm[:], lhsT=sel_sb[:], rhs=Hpack[:], start=True, stop=True
            )
            HmS = sb.tile([J, S + 1], F32, tag="HmS")
            nc.vector.tensor_copy(out=HmS[:], in_=Hm[:])
            U = Hm[:, 0:S]
            Sg2 = Hm[:, S : S + 1]

            Usq = sb.tile([J, S], F32, tag="Usq")
            Q = sb.tile([J, 1], F32, tag="Q")
            nc.vector.tensor_mul(out=Usq[:], in0=U, in1=HmS[:, 0:S])
            nc.vector.reduce_sum(out=Q[:], in_=Usq[:], axis=mybir.AxisListType.X)

            # stat_in columns: 0=push rowsum, 1=pull_v, 2=ds_v
            stat_in = sb.tile([J, 3], F32, tag="stat_in")

            # ---- pull: (Sg2 - Q*recip) * recip * valid / S ----
            na = sb.tile([J, 1], F32, tag="na")
            nc.vector.tensor_scalar(
                out=na[:], in0=Q[:], scalar1=nr[:], scalar2=None,
                op0=mybir.AluOpType.mult,
            )
            b = sb.tile([J, 1], F32, tag="b")
            nc.vector.tensor_add(out=b[:], in0=na[:], in1=Sg2)
            nc.vector.tensor_scalar(
                out=stat_in[:, 1:2], in0=b[:], scalar1=rrv[:], scalar2=None,
                op0=mybir.AluOpType.mult,
            )

            # ---- scale: valid * (1 - A*rB*rC) ----
            absU = sb.tile([J, S], F32, tag="absU")
            nc.scalar.activation(out=absU[:], in_=U, func=AF.Abs)
            rA = sb.tile([J, S], F32, tag="rA")
            A = sb.tile([J, 1], F32, tag="A")
            nc.vector.tensor_mul(out=rA[:], in0=r[:], in1=absU[:])
            nc.vector.reduce_sum(out=A[:], in_=rA[:], axis=mybir.AxisListType.X)
            mQ = sb.tile([J, 1], F32, tag="mQ")
            nc.vector.tensor_scalar_max(out=mQ[:], in0=Q[:], scalar1=1e-24)
            sC = sb.tile([J, 1], F32, tag="sC")
            nc.scalar.sqrt(out=sC[:], in_=mQ[:])
            rC = sb.tile([J, 1], F32, tag="rC")
            nc.vector.reciprocal(out=rC[:], in_=sC[:])
            d12 = sb.tile([J, 1], F32, tag="d12")
            nc.vector.tensor_scalar(
                out=d12[:], in0=A[:], scalar1=rB[:], scalar2=rC[:],
                op0=mybir.AluOpType.mult, op1=mybir.AluOpType.mult,
            )
            tds = sb.tile([J, 1], F32, tag="tds")
            nc.vector.tensor_scalar(
                out=tds[:], in0=d12[:], scalar1=valid[:], scalar2=None,
                op0=mybir.AluOpType.mult,
            )
            nc.vector.tensor_sub(out=stat_in[:, 2:3], in0=valid[:], in1=tds[:])

            # ---- push: exp(-||mean_i - mean_j||^2) over valid same-image pairs ----
            nc.vector.tensor_scalar(
                out=Wb[:, 0:S], in0=U, scalar1=recip[:], scalar2=None,
                op0=mybir.AluOpType.mult,
            )
            nc.vector.tensor_scalar(
                out=Wa[:, 0:S], in0=U, scalar1=nr[:], scalar2=None,
                op0=mybir.AluOpType.mult,
            )
            Qm = sb.tile([J, 1], F32, tag="Qm")
            nc.vector.tensor_scalar(
                out=Qm[:], in0=Q[:], scalar1=recip2[:], scalar2=None,
                op0=mybir.AluOpType.mult,
            )
            h = sb.tile([J, 1], F32, tag="h")
            nc.vector.tensor_scalar(
                out=h[:], in0=Qm[:], scalar1=0.5, scalar2=hv[:],
                op0=mybir.AluOpType.mult, op1=mybir.AluOpType.add,
            )
            nc.vector.tensor_copy(out=Wa[:, S : S + 1], in_=h[:])
            nc.vector.tensor_copy(out=Wb[:, S + 1 : S + 2], in_=h[:])

            Xp = ps.tile([18, J], F32, tag="Xp")
            nc.tensor.transpose(out=Xp[:], in_=Wa[:], identity=ident_sb)
            Yp = ps.tile([18, J], F32, tag="Yp")
            nc.tensor.transpose(out=Yp[:], in_=Wb[:], identity=ident_sb)
            X = sb.tile([18, J], F32, tag="X")
            nc.vector.tensor_copy(out=X[:], in_=Xp[:])
            Y = sb.tile([18, J], F32, tag="Y")
            nc.vector.tensor_copy(out=Y[:], in_=Yp[:])

            Dhat = ps.tile([J, J], F32, tag="Dhat")
            nc.tensor.matmul(out=Dhat[:], lhsT=X[:], rhs=Y[:], start=True, stop=True)

            Dmask = sb.tile([J, J], F32, tag="Dmask")
            nc.vector.tensor_add(out=Dmask[:], in0=maskp_sb, in1=Dhat[:])
            epx = sb.tile([J, J], F32, tag="epx")
            nc.scalar.activation(
                out=epx[:], in_=Dmask[:], func=AF.Exp, scale=-2.0,
                accum_out=stat_in[:, 0:1],
            )

            # ---- per-image reduction + finalize ----
            stats_ps = ps.tile([n_img, 3], F32, tag="stats_ps")
            nc.tensor.matmul(
                out=stats_ps[:], lhsT=ind2_sb, rhs=stat_in[:], start=True, stop=True
            )
            fs = sb.tile([n_img, 3], F32, tag="fs")
            nc.vector.tensor_copy(out=fs[:], in_=stats_ps[:])

            outbuf = sb.tile([n_img, 3], F32, tag="outbuf")
            nc.vector.tensor_scalar_mul(
                out=outbuf[:, 0:3:2], in0=fs[:, 1:3], scalar1=rn[:]
            )
            t6 = sb.tile([n_img, 1], F32, tag="t6")
            nc.vector.tensor_scalar(
                out=t6[:], in0=fs[:, 0:1], scalar1=0.5, scalar2=rp[:],
                op0=mybir.AluOpType.mult, op1=mybir.AluOpType.mult,
            )
            nc.vector.tensor_mul(out=outbuf[:, 1:2], in0=t6[:], in1=ge2[:])

            nc.sync.dma_start(out.ap(), outbuf[:])

    nc.compile()
    return nc


def make_in_map(tags, joints, box_scales, scale_dist, n_img=2):
    """Per-core input map from the core's shard (numpy views of full inputs).

    joints are re-laid-out into the kernel's doubled-partition format as part
    of sharding: [128, 18] int32, partitions 0-59 = (loc,vis) pairs of joints
    0-8, partitions 64-123 = joints 9-16, with loc rebased into the shard's
    flattened [n_img*L, 16] coordinate system (+ img*L).
    """
    J = n_img * M
    sd = np.asarray(scale_dist, dtype=np.float32).reshape(1, S)
    bs = np.concatenate(
        [np.asarray(box_scales, np.float32).reshape(J, 1), np.tile(sd, (J, 1))], axis=1
    )
    jr = np.asarray(joints).reshape(J, K, 2).astype(np.int32)
    offs = (np.arange(J) // M * L).astype(np.int32)
    jr = jr.copy()
    jr[:, :, 0] += offs[:, None]
    j2 = np.zeros((128, 18), np.int32)
    j2[0:J, :] = jr[:, 0:9, :].reshape(J, 18)
    j2[64 : 64 + J, 0:16] = jr[:, 9:17, :].reshape(J, 16)
    return {
        "tags": np.ascontiguousarray(tags.reshape(n_img * L, S), dtype=np.float32),
        "joints": j2,
        "bs": np.ascontiguousarray(bs),
    }

_NC_CACHE = {}


def _get_nc():
    if "nc" not in _NC_CACHE:
        _NC_CACHE["nc"] = build_nc()
    return _NC_CACHE["nc"]


def kernel(tags, joints, box_scales, scale_dist, _trace=False):
    """Full-input entry point; shards across 8 NeuronCores and gathers."""
    tags = np.asarray(tags)
    joints = np.asarray(joints)
    box_scales = np.asarray(box_scales)
    scale_dist = np.asarray(scale_dist)

    nc = _get_nc()
    in_maps = [
        make_in_map(
            tags[N_IMG * c : N_IMG * (c + 1)],
            joints[N_IMG * c : N_IMG * (c + 1)],
            box_scales[N_IMG * c : N_IMG * (c + 1)],
            scale_dist,
        )
        for c in range(N_CORES)
    ]
    res = run_bass_kernel_spmd(
        nc, in_maps, core_ids=list(range(N_CORES)), trace=_trace
    )
    parts = np.concatenate(
        [res.results[c]["out"] for c in range(N_CORES)], axis=0
    )  # [N, 3]
    final = parts.mean(axis=0).astype(np.float32)
    if _trace:
        return final, res
    return final



# revision 4
# speedup vs baseline: 1.2047x; 1.2047x over previous
"""AssociativeEmbeddingLoss on 8 TRN2 NeuronCores (Bass/Tile kernel) — v3.

Entry point: kernel(**inputs) -> np.ndarray (3,) = (pull, push, scale),
matching the reference. Data-parallel on batch dim N=16 -> 2 images per
core; per-image partials are averaged on the host.

Design (vs the 39.4us 9-call baseline):
  - Only VISIBLE joints are gathered: invisible ones are multiplied by
    zero downstream anyway, so the host compacts the ~1020 (person,
    joint) pairs to the ~510 visible ones. 640 descriptor capacity
    (8 sigma above the Binomial(1020,1/2) mean) -> FIVE indirect-DMA
    calls instead of nine; a >640 overflow (never for random inputs)
    falls back to a lazily-built 9-call variant.
  - The visibility mask is folded into per-block 0/1 selector matrices:
    as each 128-descriptor gather block lands, one PE matmul
    accumulates both U = sum(vis*g) and V = sum(vis*g^2) into PSUM
    ([64,32], rhs = [G_c | G_c^2] via a strided view), entirely hidden
    under the remaining gather stream. No 128->60 half-merge needed.
  - Everything derivable from visibility counts / box_scales alone
    (reciprocals, valid masks, normalized scale targets tgt, per-image
    1/n factors) is precomputed on the host into one constant-block
    DMA. The device only computes what needs gathered tag values.
  - All activations used (Exp/Abs) live in one activation-table set, so
    one hidden table load replaces the baseline's five (two of which
    sat on the critical path).
  - rsqrt(||U||^2) for the scale loss is a DVE Newton iteration seeded
    by the int32 bit trick - no Sqrt table set, no Act round-trip.
  - The push-loss pair mask folds into the Gram matmul via +-64.0
    image-indicator feature columns (64^2=4096 exactly cancels the 2048
    h-constants in fp32); the diagonal exp(0)=1 surplus per valid
    person is cancelled by host-constant pseudo-rows in the final
    per-image reduction matmul.
"""

import numpy as np

import concourse.bacc as bacc
import concourse.mybir as mybir
import concourse.tile as tile
from concourse.bass import IndirectOffsetOnAxis
from concourse.bass_utils import run_bass_kernel_spmd

F32 = mybir.dt.float32
I32 = mybir.dt.int32
AF = mybir.ActivationFunctionType
ALU = mybir.AluOpType

S = 16      # scale-embedding dim
K = 17      # joints
M = 30      # persons per image
N = 16      # batch
L = 69632   # flattened tag locations per image
N_CORES = 8
N_IMG = N // N_CORES    # images per core
JR = 64                 # person rows per core (2 images x 32, rows 30/31 dead)
CB = 64.0               # c; c^2 = 4096 exact

# feature flags (validated by HW probes; flip off to use safe fallbacks)
USE_NEWTON = True       # DVE bit-trick rsqrt instead of Sqrt activation table
USE_TTR = False         # fused tensor_tensor_reduce: custom-DVE op, crashes this runtime
USE_STT = True          # fused scalar_tensor_tensor

# TLC (constant-block) column layout, [66, TLC_W]
C_IDENT = 0      # 0:64 identity for the PE transpose
C_IND = 64       # 64:66 image one-hot; rows 64/65 = eye(2) pseudo
C_WA = 68        # 68:88  Wa: [-mean(16) | h | 1 | c*ind(2)]
C_WB = 100       # 100:120 Wb: [ mean(16) | 1 | h | -c*ind(2)]
C_TGT = 132      # 132:148 normalized scale target
C_RECIP = 148
C_NR = 149
C_RRVP = 150
C_H0 = 151
C_HV2 = 152
C_VRN = 153
C_CPUSH = 154
C_NVRN = 155
C_STAT = 156     # 156:159 stat cols: pull | push | scale; rows 64/65 pseudo
TLC_W = 160


def build_nc(n_blk=5):
    """n_blk gather blocks of 128 descriptors each."""
    nc = bacc.Bacc("TRN2", target_bir_lowering=False, debug=False)

    tags = nc.dram_tensor("tags", [N_IMG * L, S], F32, kind="ExternalInput")
    j2d = nc.dram_tensor("j2", [128, n_blk], I32, kind="ExternalInput")
    seld = nc.dram_tensor("sel", [128, n_blk * JR], F32, kind="ExternalInput")
    tlcd = nc.dram_tensor("tlc", [JR + 2, TLC_W], F32, kind="ExternalInput")
    out = nc.dram_tensor("out", [N_IMG, 3], F32, kind="ExternalOutput")

    with tile.TileContext(nc) as tc:
        with (
            tc.tile_pool(name="sb", bufs=1) as sb,
            tc.tile_pool(name="ps", bufs=1, space="PSUM") as ps,
        ):
            # ---- loads. j2 gates the gather stream -> first on Act queue;
            # sel on the idle sync queue; one hidden ACT table load (Exp
            # set, which also contains Abs) via the warmup activation ----
            j2 = sb.tile([128, n_blk], I32, tag="j2")
            nc.scalar.dma_start(j2[:], j2d.ap())
            tlc = sb.tile([JR + 2, TLC_W], F32, tag="tlc")
            nc.scalar.dma_start(tlc[:], tlcd.ap())
            sel = sb.tile([128, n_blk * JR], F32, tag="sel")
            nc.sync.dma_start(sel[:], seld.ap())

            warm = sb.tile([1, 2], F32, tag="warm")
            nc.vector.memset(warm[:, 0:1], 1.0)
            nc.scalar.activation(out=warm[:, 1:2], in_=warm[:, 0:1], func=AF.Exp)

            # ---- gather stream + per-block accumulation ----
            # GS cols [0 : 16*n_blk) = gathered rows, [16*n_blk : 32*n_blk)
            # = their squares; one PE matmul per block accumulates
            # [U | V] = sel_c^T @ [G_c | G_c^2] into PSUM.
            GS = sb.tile([128, 2 * S * n_blk], F32, tag="GS")
            uv = ps.tile([JR, 2 * S], F32, tag="uv")
            for c in range(n_blk):
                nc.gpsimd.indirect_dma_start(
                    out=GS[:, c * S : (c + 1) * S],
                    out_offset=None,
                    in_=tags.ap(),
                    in_offset=IndirectOffsetOnAxis(ap=j2[:, c : c + 1], axis=0),
                )
            sqo = n_blk * S
            for c in range(n_blk):
                g_c = GS[:, c * S : (c + 1) * S]
                nc.vector.tensor_mul(
                    out=GS[:, sqo + c * S : sqo + (c + 1) * S], in0=g_c, in1=g_c
                )
                rhs = GS[:].rearrange("p (b c s) -> p b c s", b=2, c=n_blk)[
                    :, :, c : c + 1, :
                ]
                nc.tensor.matmul(
                    out=uv[:],
                    lhsT=sel[:, c * JR : (c + 1) * JR],
                    rhs=rhs,
                    start=(c == 0),
                    stop=(c == n_blk - 1),
                )

            # ---- per-person stats ----
            tj = tlc[0:JR, :]
            U2 = sb.tile([JR, 2 * S], F32, tag="U2")
            nc.vector.tensor_copy(out=U2[:], in_=uv[:])
            sg = sb.tile([JR, 1], F32, tag="sg")
            nc.vector.reduce_sum(
                out=sg[:], in_=U2[:, S : 2 * S], axis=mybir.AxisListType.X
            )
            q = sb.tile([JR, 1], F32, tag="q")
            scr16 = sb.tile([JR, S], F32, tag="scr16")
            if USE_TTR:
                nc.vector.tensor_tensor_reduce(
                    out=scr16[:], in0=U2[:, 0:S], in1=U2[:, 0:S], scale=1.0,
                    scalar=0.0, op0=ALU.mult, op1=ALU.add, accum_out=q[:],
                )
            else:
                nc.vector.tensor_mul(out=scr16[:], in0=U2[:, 0:S], in1=U2[:, 0:S])
                nc.vector.reduce_sum(
                    out=q[:], in_=scr16[:], axis=mybir.AxisListType.X
                )

            # push feature data columns (consts arrived via the TLC DMA)
            nc.vector.tensor_scalar_mul(
                out=tj[:, C_WB : C_WB + S], in0=U2[:, 0:S],
                scalar1=tj[:, C_RECIP : C_RECIP + 1],
            )
            nc.vector.tensor_scalar_mul(
                out=tj[:, C_WA : C_WA + S], in0=U2[:, 0:S],
                scalar1=tj[:, C_NR : C_NR + 1],
            )
            nc.vector.tensor_scalar(
                out=tj[:, C_WA + S : C_WA + S + 1], in0=q[:],
                scalar1=tj[:, C_H0 : C_H0 + 1], scalar2=tj[:, C_HV2 : C_HV2 + 1],
                op0=ALU.mult, op1=ALU.add,
            )
            nc.vector.tensor_copy(
                out=tj[:, C_WB + S + 1 : C_WB + S + 2],
                in_=tj[:, C_WA + S : C_WA + S + 1],
            )

            absU = sb.tile([JR, S], F32, tag="absU")
            nc.scalar.activation(out=absU[:], in_=U2[:, 0:S], func=AF.Abs)

            # ---- push: one transpose, two aligned copies, Gram, Exp ----
            tp = ps.tile([64, JR], F32, tag="tp")
            nc.tensor.transpose(
                out=tp[:], in_=tj[:, C_WA : C_WA + 64], identity=tj[:, 0:JR]
            )
            X = sb.tile([20, JR], F32, tag="X")
            nc.vector.tensor_copy(out=X[:], in_=tp[0:20, :])
            Y = sb.tile([20, JR], F32, tag="Y")
            nc.vector.tensor_copy(out=Y[:], in_=tp[32:52, :])
            dh = ps.tile([JR, JR], F32, tag="dh")
            nc.tensor.matmul(out=dh[:], lhsT=X[:], rhs=Y[:], start=True, stop=True)

            # ---- scale branch: rsqrt(q) ----
            rq = sb.tile([JR, 1], F32, tag="rq")
            mq = sb.tile([JR, 1], F32, tag="mq")
            nc.vector.tensor_scalar_max(out=mq[:], in0=q[:], scalar1=1e-30)
            if USE_NEWTON:
                ti = sb.tile([JR, 1], I32, tag="ti")
                nc.vector.tensor_single_scalar(
                    out=ti[:], in_=mq[:].bitcast(I32), scalar=1,
                    op=ALU.logical_shift_right,
                )
                yi = sb.tile([JR, 1], I32, tag="yi")
                nc.vector.tensor_scalar(
                    out=yi[:], in0=ti[:], scalar1=-1, scalar2=0x5F3759DF,
                    op0=ALU.mult, op1=ALU.add,
                )
                y0 = yi[:].bitcast(F32)
                y2 = sb.tile([JR, 1], F32, tag="y2")
                e = sb.tile([JR, 1], F32, tag="e")
                f = sb.tile([JR, 1], F32, tag="f")
                y1 = sb.tile([JR, 1], F32, tag="y1")
                nc.vector.tensor_mul(out=y2[:], in0=y0, in1=y0)
                nc.vector.tensor_mul(out=e[:], in0=mq[:], in1=y2[:])
                nc.vector.tensor_scalar(
                    out=f[:], in0=e[:], scalar1=-0.5, scalar2=1.5,
                    op0=ALU.mult, op1=ALU.add,
                )
                nc.vector.tensor_mul(out=y1[:], in0=y0, in1=f[:])
                nc.vector.tensor_mul(out=y2[:], in0=y1[:], in1=y1[:])
                nc.vector.tensor_mul(out=e[:], in0=mq[:], in1=y2[:])
                nc.vector.tensor_scalar(
                    out=f[:], in0=e[:], scalar1=-0.5, scalar2=1.5,
                    op0=ALU.mult, op1=ALU.add,
                )
                nc.vector.tensor_mul(out=rq[:], in0=y1[:], in1=f[:])
            else:
                sq_ = sb.tile([JR, 1], F32, tag="sq_")
                nc.scalar.sqrt(out=sq_[:], in_=mq[:])
                nc.vector.reciprocal(out=rq[:], in_=sq_[:])

            A = sb.tile([JR, 1], F32, tag="A")
            if USE_TTR:
                nc.vector.tensor_tensor_reduce(
                    out=scr16[:], in0=absU[:], in1=tj[:, C_TGT : C_TGT + S],
                    scale=1.0, scalar=0.0, op0=ALU.mult, op1=ALU.add,
                    accum_out=A[:],
                )
            else:
                nc.vector.tensor_mul(
                    out=scr16[:], in0=absU[:], in1=tj[:, C_TGT : C_TGT + S]
                )
                nc.vector.reduce_sum(
                    out=A[:], in_=scr16[:], axis=mybir.AxisListType.X
                )
            d12 = sb.tile([JR, 1], F32, tag="d12")
            nc.vector.tensor_mul(out=d12[:], in0=A[:], in1=rq[:])
            # scale stat = vrn - d12*vrn
            if USE_STT:
                nc.vector.scalar_tensor_tensor(
                    out=tj[:, C_STAT + 2 : C_STAT + 3], in0=d12[:],
                    scalar=tj[:, C_NVRN : C_NVRN + 1],
                    in1=tj[:, C_VRN : C_VRN + 1],
                    op0=ALU.mult, op1=ALU.add,
                )
            else:
                t2 = sb.tile([JR, 1], F32, tag="t2")
                nc.vector.tensor_scalar(
                    out=t2[:], in0=d12[:], scalar1=tj[:, C_NVRN : C_NVRN + 1],
                    scalar2=None, op0=ALU.mult,
                )
                nc.vector.tensor_scalar(
                    out=tj[:, C_STAT + 2 : C_STAT + 3], in0=t2[:],
                    scalar1=tj[:, C_VRN : C_VRN + 1], scalar2=None, op0=ALU.add,
                )
            # pull stat = (q*nr + sg) * rrvp
            p1 = sb.tile([JR, 1], F32, tag="p1")
            if USE_STT:
                nc.vector.scalar_tensor_tensor(
                    out=p1[:], in0=q[:], scalar=tj[:, C_NR : C_NR + 1],
                    in1=sg[:], op0=ALU.mult, op1=ALU.add,
                )
            else:
                t3 = sb.tile([JR, 1], F32, tag="t3")
                nc.vector.tensor_scalar(
                    out=t3[:], in0=q[:], scalar1=tj[:, C_NR : C_NR + 1],
                    scalar2=None, op0=ALU.mult,
                )
                nc.vector.tensor_add(out=p1[:], in0=t3[:], in1=sg[:])
            nc.vector.tensor_scalar_mul(
                out=tj[:, C_STAT : C_STAT + 1], in0=p1[:],
                scalar1=tj[:, C_RRVP : C_RRVP + 1],
            )

            # push stat: row-sums of exp(-2*Gram), scaled by cpush
            eo = sb.tile([JR, JR], F32, tag="eo")
            ea = sb.tile([JR, 1], F32, tag="ea")
            nc.scalar.activation(
                out=eo[:], in_=dh[:], func=AF.Exp, scale=-2.0, accum_out=ea[:]
            )
            nc.vector.tensor_scalar_mul(
                out=tj[:, C_STAT + 1 : C_STAT + 2], in0=ea[:],
                scalar1=tj[:, C_CPUSH : C_CPUSH + 1],
            )

            # ---- per-image reduction (pseudo-rows fold the diagonal fix) ----
            fsp = ps.tile([N_IMG, 3], F32, tag="fsp")
            nc.tensor.matmul(
                out=fsp[:], lhsT=tlc[:, C_IND : C_IND + 2],
                rhs=tlc[:, C_STAT : C_STAT + 3], start=True, stop=True,
            )
            ob = sb.tile([N_IMG, 3], F32, tag="ob")
            nc.vector.tensor_copy(out=ob[:], in_=fsp[:])
            nc.sync.dma_start(out.ap(), ob[:])

    nc.compile()
    return nc


def _prep_core(tags_c, joints_c, box_c, sd, n_blk):
    """Host-side shard prep: compacted visible-joint gather list, per-block
    selectors, and the constant block. Returns None if the visible count
    exceeds this build's capacity (caller rebuilds with more blocks)."""
    cap = 128 * n_blk
    tags2 = np.ascontiguousarray(
        np.asarray(tags_c, dtype=np.float32).reshape(N_IMG * L, S)
    )
    jl = np.asarray(joints_c[..., 0], dtype=np.int64)      # [2, 30, 17]
    vis = np.asarray(joints_c[..., 1]) > 0
    loc = (jl + (np.arange(N_IMG) * L)[:, None, None]).astype(np.int64)

    img_r, m_r, k_r = np.nonzero(vis)          # visible (img, person, joint)
    V = img_r.shape[0]
    if V > cap:
        return None
    jrow = img_r * 32 + m_r                    # person row 0..63
    locv = loc[img_r, m_r, k_r].astype(np.int32)

    j2 = np.zeros((128, n_blk), np.int32)
    selm = np.zeros((128, n_blk * JR), np.float32)
    fi = np.arange(V)
    p_i, c_i = fi % 128, fi // 128
    j2[p_i, c_i] = locv
    selm[p_i, c_i * JR + jrow] = 1.0

    visf = vis.reshape(N_IMG * M, K).astype(np.float32)
    cnt_pm = visf.sum(1).reshape(N_IMG, M)     # [2, 30]
    cnt = np.zeros((N_IMG, 32), np.float32)
    cnt[:, 0:M] = cnt_pm
    cnt = cnt.reshape(JR)
    recip = (1.0 / np.maximum(cnt, 1.0)).astype(np.float32)
    valid = (cnt > 0).astype(np.float32)
    imgr = np.arange(JR) // 32
    nv = np.array([valid[imgr == i].sum() for i in range(N_IMG)], np.float32)
    rn = (1.0 / np.maximum(nv, 1.0)).astype(np.float32)
    rp = (1.0 / np.maximum(nv * (nv - 1.0), 1.0)).astype(np.float32)
    ge2 = (nv >= 2.0).astype(np.float32)
    cpush = 0.5 * rp * ge2

    box = np.zeros((N_IMG, 32), np.float32)
    box[:, 0:M] = np.asarray(box_c, dtype=np.float32).reshape(N_IMG, M)
    box = box.reshape(JR)
    sd = np.asarray(sd, dtype=np.float32).reshape(S)
    gap = np.abs(box[:, None] - sd[None, :]).astype(np.float32)
    r = (np.float32(1.0) / (gap + np.float32(1e-10))).astype(np.float32)
    nrm = np.sqrt((r * r).sum(1, dtype=np.float32))
    tgt = r / np.maximum(nrm, np.float32(1e-12))[:, None]

    tlc = np.zeros((JR + 2, TLC_W), np.float32)
    pj = np.arange(JR)
    tlc[0:JR, 0:JR] = np.eye(JR, dtype=np.float32)
    tlc[pj, C_IND + imgr] = 1.0
    tlc[JR, C_IND] = 1.0
    tlc[JR + 1, C_IND + 1] = 1.0
    tlc[0:JR, C_WA + S + 1] = 1.0
    tlc[pj, C_WA + S + 2 + imgr] = CB
    tlc[0:JR, C_WB + S] = 1.0
    tlc[pj, C_WB + S + 2 + imgr] = -CB
    tlc[0:JR, C_TGT : C_TGT + S] = tgt
    tlc[0:JR, C_RECIP] = recip
    tlc[0:JR, C_NR] = -recip
    tlc[0:JR, C_RRVP] = (recip / S) * valid * rn[imgr]
    tlc[0:JR, C_H0] = 0.5 * recip * recip
    tlc[0:JR, C_HV2] = 4096.0 * (1.0 - valid) + 2048.0
    tlc[0:JR, C_VRN] = valid * rn[imgr]
    tlc[0:JR, C_CPUSH] = cpush[imgr]
    tlc[0:JR, C_NVRN] = -tlc[0:JR, C_VRN]
    tlc[JR, C_STAT + 1] = -cpush[0] * nv[0]
    tlc[JR + 1, C_STAT + 1] = -cpush[1] * nv[1]
    return {"tags": tags2, "j2": j2, "sel": selm, "tlc": tlc}


_NC_CACHE = {}


def _get_nc(n_blk):
    if n_blk not in _NC_CACHE:
        _NC_CACHE[n_blk] = build_nc(n_blk)
    return _NC_CACHE[n_blk]


def kernel(tags, joints, box_scales, scale_dist, _trace=False):
    """Full-input entry point; shards across 8 NeuronCores and gathers."""
    tags = np.asarray(tags)
    joints = np.asarray(joints)
    box_scales = np.asarray(box_scales)
    scale_dist = np.asarray(scale_dist)

    for n_blk in (5, 9):  # 9-block fallback only if >640 joints are visible
        in_maps = [
            _prep_core(
                tags[N_IMG * c : N_IMG * (c + 1)],
                joints[N_IMG * c : N_IMG * (c + 1)],
                box_scales[N_IMG * c : N_IMG * (c + 1)],
                scale_dist,
                n_blk,
            )
            for c in range(N_CORES)
        ]
        if all(m is not None for m in in_maps):
            break

    res = run_bass_kernel_spmd(
        _get_nc(n_blk), in_maps, core_ids=list(range(N_CORES)), trace=_trace
    )
    parts = np.concatenate(
        [res.results[c]["out"] for c in range(N_CORES)], axis=0
    )  # [N, 3]
    final = parts.mean(axis=0).astype(np.float32)
    if _trace:
        return final, res
    return final
